# revision 46
# baseline (speedup 1.0000x reference)
"""Trainium2 Bass kernel for implicit cross-attention (keys/values = queries + 1 ctx token).

Sharding: 8 cores = 4 batches x 2 head-groups (8 heads each). Each core computes
q = x_b @ Wq[:, g], causal flash-style attention over keys [ctx, q_0..q_{N-1}],
and a partial output projection out @ Wo[g, :]. Host sums the two head-group
partials per batch and adds the bias.

Final version (~260-265 us HW, vs 361 us staged baseline). Key scheduling
decisions, all verified against neuron-profile traces:
- input DMAs split in halves and striped over 3 engine queues
  (sync/scalar/gpsimd), issued before any gpsimd memsets (which otherwise
  block the gpsimd queue's DMA triggers for ~28 us).
- ctx score rows (sc8 -> pcx8) interleaved into the q-projection loop per
  4-block chunk; no serial tail before attention.
- attention kb loop software-pipelined with a 1-block score lookahead:
  scores(kb+1) are issued to the PE before av(kb), so the PE FIFO head never
  blocks the next exp behind an av matmul that is still waiting on exp(kb).
  The attention phase runs at ~93% ACT occupancy (exp is the floor).
- ctx-token contribution built off the PE (gpsimd partition broadcast + DVE
  per-partition scale) and folded into the U evacuation add; first av matmul
  uses start=True. This removes the K=1 seed matmuls that gated every pair
  transition.
- loop order w-outer / m-inner; window-0 output projection is interleaved
  into window-1 attention at pair boundaries, reusing the pu PSUM tags
  (2 banks) that have just been evacuated. The tail rotates py over 4 PSUM
  slots (pu tags + freed sp slots); held-back window-0 blocks keep the PE's
  HAM clock warm through the final normalization chain; y is written bf16
  (host upcasts when summing the two head-group partials).
- normalization: denominator extract DMA -> DVE reciprocal in place ->
  gpsimd partition broadcast -> DVE multiply, software-pipelined 4 deep.
Measured dead ends (do not retry): K=64 row-tiled score matmuls via
tile_position serialize AND drop the PE to its throttled 1.2 GHz clock for
the whole attention phase; a post-exp DVE causal mask puts the DVE on the
exp->av critical path (keep the trineg mask matmul inside the score
accumulation group); PSUM (8 banks) cannot fit combined-head exp tiles
(sp 2x[128,2048] + pu 2x[128,1024] = 12 banks), which blocks halving the
exp instruction-overhead.
"""

import numpy as np
import ml_dtypes

import concourse.bass as bass
import concourse.mybir as mybir
from concourse import bacc
from concourse.tile import TileContext
from concourse.bass_utils import run_bass_kernel_spmd

FP = mybir.dt.float32
BF = mybir.dt.bfloat16

N = 2048          # sequence length
CD = 1024         # model dim
HD = 512          # head-dim cols per core (8 heads x 64)
D = 64            # dim per head
NHEAD = 8         # heads per core
NPAIR = 4         # head pairs (2 heads share a 128-partition tile)
SCALE = 0.125     # D ** -0.5
NCC = CD // 128   # 8 contraction chunks
NQB = N // 128    # 16 query/key blocks of 128
NW = N // 1024    # 2 query windows of 1024
WQ = 1024         # window width


def _build_nc():
    nc = bacc.Bacc("TRN2", target_bir_lowering=False)
    xt_d = nc.declare_dram_parameter("xt", [CD, N], BF, isOutput=False)
    wq_d = nc.declare_dram_parameter("wq", [CD, HD], BF, isOutput=False)
    wo_d = nc.declare_dram_parameter("wo", [HD, CD], BF, isOutput=False)
    kctx_d = nc.declare_dram_parameter("kctx", [1, HD], FP, isOutput=False)
    vctx_d = nc.declare_dram_parameter("vctx", [1, HD], FP, isOutput=False)
    y_d = nc.declare_dram_parameter("y", [N, CD], BF, isOutput=True)

    with TileContext(nc) as tc, tc.tile_pool(name="persist", bufs=1) as pp:
        # ---- persistent SBUF tensors ----
        ones11 = pp.tile([1, 1], FP, tag="ones11", name="ones11")
        trinegT = pp.tile([128, 128], BF, tag="trinegT", name="trinegT")
        qkT = [pp.tile([128, N], BF, tag=f"qkT{m}", name=f"qkT{m}") for m in range(NPAIR)]
        # v + ones column per head (U stationary), 65-stride layout, padded so
        # a 128-wide stationary window exists for the last head
        v65 = [pp.tile([128, NHEAD * (D + 1) + D], BF, tag=f"v65_{b}", name=f"v65_{b}")
               for b in range(NQB)]
        # zero-banded per-head q^T copies (K=128 score stationaries)
        qkZ = [pp.tile([128, N], BF, tag=f"qkZ{h}", name=f"qkZ{h}") for h in range(NHEAD)]
        attnT = [pp.tile([128, N], BF, tag=f"attnT{m}", name=f"attnT{m}") for m in range(NPAIR)]
        wq_sb = [pp.tile([128, HD], BF, tag=f"wq{c}", name=f"wq{c}") for c in range(NCC)]
        wo_sb = [pp.tile([128, CD], BF, tag=f"wo{m}", name=f"wo{m}") for m in range(NPAIR)]
        kctx_sb = pp.tile([1, HD], FP, tag="kctx", name="kctx")
        vctx_sb = pp.tile([1, HD], FP, tag="vctxr", name="vctxr")
        kct_sb = pp.tile([64, NHEAD], FP, tag="kct", name="kct")
        # per-pair zero-masked bf16 k_ctx^T columns (for accumulated ctx scores)
        kct2z = [pp.tile([128, NHEAD], BF, tag=f"kct2z{m}", name=f"kct2z{m}")
                 for m in range(NPAIR)]
        # v_ctx per head as a per-partition column [65, h]: rows 0:64 = dims,
        # row 64 = 1 (denominator slot)
        vct65 = pp.tile([65, NHEAD], FP, tag="vct65", name="vct65")
        pcx8 = pp.tile([8, N], BF, tag="pcx8", name="pcx8")

        identb = pp.tile([128, 128], BF, tag="identb", name="identb")

        with tc.tile_pool(name="xt", bufs=1) as xt_pool, \
             tc.tile_pool(name="qp", bufs=2, space="PSUM") as qp_pool, \
             tc.tile_pool(name="scp", bufs=1, space="PSUM") as scp_pool:
            xT = [xt_pool.tile([128, N], BF, tag=f"xT{c}", name=f"xT{c}") for c in range(NCC)]
            vsb = [xt_pool.tile([128, HD], BF, tag=f"vsb{b}", name=f"vsb{b}")
                   for b in range(NQB)]
            # ---- input DMAs first (before any memsets occupy the queues),
            # striped over 3 queues; x^T pre-transposed on host ----
            dq = [nc.sync, nc.scalar, nc.gpsimd]
            nc.sync.dma_start(kctx_sb, kctx_d[0:1, :])
            nc.scalar.dma_start(vctx_sb, vctx_d[0:1, :])
            for c in range(NCC):
                rsl = slice(128 * c, 128 * (c + 1))
                dq[(2 * c) % 3].dma_start(xT[c][:, 0:1024], xt_d[rsl, 0:1024])
                dq[(2 * c + 1) % 3].dma_start(xT[c][:, 1024:2048], xt_d[rsl, 1024:2048])
                dq[(2 * c + 2) % 3].dma_start(wq_sb[c], wq_d[rsl, :])
            for m in range(NPAIR):
                dq[m % 3].dma_start(wo_sb[m], wo_d[128 * m:128 * (m + 1), :])

            # HAM warm-up: ~30 dummy matmuls during the otherwise-dead DMA
            # wait flip the PE clock gate to 8/8 before real work arrives
            warm = xt_pool.tile([128, 128], BF, tag="warm", name="warm")
            nc.vector.memset(warm, 0.001)
            wps = qp_pool.tile([128, 512], FP, tag="qp", name="wps")
            for _ in range(30):
                nc.tensor.matmul(wps[:, 0:128], warm, warm,
                                 start=True, stop=True, skip_group_check=True)

            nc.vector.memset(ones11, 1.0)
            nc.gpsimd.memset(identb, 0.0)
            nc.gpsimd.affine_select(
                out=identb, in_=identb, compare_op=mybir.AluOpType.not_equal,
                fill=1.0, base=0, pattern=[[-1, 128]], channel_multiplier=1)
            nc.gpsimd.memset(trinegT, 0.0)
            # lhsT for the mask matmul: -1e30 where p < f (strict upper
            # triangle), so (trinegT.T @ I)[k, j] = -1e30 for j < k
            nc.gpsimd.affine_select(
                out=trinegT, in_=trinegT, compare_op=mybir.AluOpType.is_ge,
                fill=-1e30, base=0, pattern=[[-1, 128]], channel_multiplier=1)
            for b in range(NQB):
                nc.gpsimd.memset(v65[b], 1.0)
            for h in range(NHEAD):
                nc.gpsimd.memset(qkZ[h], 0.0)

            def emit_kct():
                # k_ctx^T / v_ctx^T per head -> kct_sb [64, 8] and vct65 [65, 8]
                kct_ps = qp_pool.tile([128, 512], FP, tag="qp", name="kctps")
                for h in range(NHEAD):
                    nc.tensor.transpose(kct_ps[0:64, h:h + 1],
                                        kctx_sb[0:1, 64 * h:64 * h + 64], ones11)
                    nc.tensor.transpose(kct_ps[0:64, 8 + h:9 + h],
                                        vctx_sb[0:1, 64 * h:64 * h + 64], ones11)
                nc.vector.tensor_copy(kct_sb, kct_ps[0:64, 0:NHEAD])
                nc.vector.tensor_copy(vct65[0:64, :], kct_ps[0:64, 8:8 + NHEAD])
                nc.vector.memset(vct65[64:65, :], 1.0)
                for m in range(NPAIR):
                    nc.gpsimd.memset(kct2z[m], 0.0)
                    nc.vector.tensor_copy(kct2z[m][0:64, 2 * m:2 * m + 1],
                                          kct_sb[:, 2 * m:2 * m + 1])
                    tmp = xt_pool.tile([64, 1], BF, tag=f"kctmp{m}", name="kctmp")
                    nc.vector.tensor_copy(tmp, kct_sb[:, 2 * m + 1:2 * m + 2])
                    nc.sync.dma_start(kct2z[m][64:128, 2 * m + 1:2 * m + 2], tmp)

            # ---- q projection: q_nat[qb] = sum_c xT[c][:, qb].T @ Wq[c].
            # The ctx score rows (sc8 -> pcx8 -> pcx_pair) and pair-0's qkZ
            # band copies are interleaved per 4-block chunk so no serial tail
            # remains before attention. ----
            sc8 = scp_pool.tile([8, N], FP, tag="sc8", name="sc8")
            for qb in range(NQB):
                qps = qp_pool.tile([128, HD], FP, tag="qp", name="qp")
                for c in range(NCC):
                    nc.tensor.matmul(qps,
                                     xT[c][:, 128 * qb:128 * (qb + 1)],
                                     wq_sb[c],
                                     start=(c == 0), stop=(c == NCC - 1))
                nc.vector.tensor_copy(vsb[qb], qps)
                # U stationary copy (v + ones col, 65-stride); ScalarE is idle
                # during this phase
                nc.scalar.copy(
                    v65[qb][:, 0:NHEAD * (D + 1)]
                        .rearrange("p (h e) -> p h e", e=D + 1)[:, :, 0:D],
                    vsb[qb].rearrange("p (h e) -> p h e", e=D))
                # q^T per pair via PE transpose (bf16, 1 cyc/row)
                tps = qp_pool.tile([128, HD], BF, tag="tps", name="tps")
                for m in range(NPAIR):
                    nc.tensor.transpose(tps[:, 128 * m:128 * (m + 1)],
                                        vsb[qb][:, 128 * m:128 * (m + 1)], identb)
                for m in range(NPAIR):
                    eng = nc.vector.tensor_copy if m % 2 == 0 else nc.scalar.copy
                    eng(qkT[m][:, 128 * qb:128 * (qb + 1)],
                        tps[:, 128 * m:128 * (m + 1)])
                if qb == 0:
                    # ctx transposes after the first q block: the PE starts on
                    # projection work instead of waiting for the tiny ctx DMAs
                    emit_kct()
                if qb % 4 == 3:
                    # ctx scores + exp + pcx replication for this 512 chunk
                    s = qb // 4
                    sl = slice(512 * s, 512 * (s + 1))
                    for m in range(NPAIR):
                        nc.tensor.matmul(sc8[:, sl], kct2z[m], qkT[m][:, sl],
                                         start=(m == 0), stop=(m == NPAIR - 1),
                                         skip_group_check=True)
                    nc.scalar.activation(pcx8[:, sl], sc8[:, sl],
                                         mybir.ActivationFunctionType.Exp,
                                         scale=SCALE)
                    # pair-0 zero-banded q^T for this chunk
                    nc.vector.tensor_copy(qkZ[0][0:64, sl], qkT[0][0:64, sl])
                    nc.vector.tensor_copy(qkZ[1][64:128, sl], qkT[0][64:128, sl])

        # ---- attention: w-outer / m-inner, flash over key blocks with a
        # 1-block score lookahead so the PE FIFO stays ahead of the ACT.
        # Scores: zero-banded K=128 matmuls + trineg mask matmul on the diag
        # block; exp on ScalarE; U (attn @ v) accumulates in PSUM with a
        # ones-column denominator row. Window-0 out-projection interleaves
        # into window-1 pair boundaries via pu-tag PSUM reuse. ----
        norm_q = []   # deferred normalization steps (software pipelining)
        oproj_q = []  # out-projection blocks ready to interleave

        def emit_norm(item):
            m, w, u8h, stage = item
            sl = slice(WQ * w, WQ * (w + 1))
            if stage == 0:
                # extract each head's denominator row to partition 0
                for hi in range(2):
                    dn = rbc_pool.tile([1, WQ], FP, tag=f"dn{hi}", name="dn")
                    nc.sync.dma_start(dn, u8h[hi]["u8"][64:65, :])
                    u8h[hi]["dn"] = dn
            elif stage == 1:
                for hi in range(2):
                    nc.vector.reciprocal_approx_fast(u8h[hi]["dn"], u8h[hi]["dn"])
            elif stage == 2:
                for hi in range(2):
                    rbc = rbc_pool.tile([64, WQ], FP, tag=f"rbc{hi}", name="rbc")
                    nc.gpsimd.partition_broadcast(rbc, u8h[hi]["dn"])
                    u8h[hi]["rbc"] = rbc
            else:
                for hi in range(2):
                    band = 64 * hi
                    nc.vector.tensor_mul(attnT[m][band:band + 64, sl],
                                         u8h[hi]["u8"][0:64, :],
                                         u8h[hi]["rbc"])

        def emit_qkz(m):
            # chunked so no single DVE op blocks the queue head for long
            for s in range(4):
                sl = slice(512 * s, 512 * (s + 1))
                nc.vector.tensor_copy(qkZ[2 * m][0:64, sl], qkT[m][0:64, sl])
                nc.vector.tensor_copy(qkZ[2 * m + 1][64:128, sl], qkT[m][64:128, sl])

        def oproj_mms(py, nb, chunk):
            # chunk i of 4: two matmuls, (co, m) pairs in co-major order
            for j in range(2):
                k = 2 * chunk + j
                co, m = k // NPAIR, k % NPAIR
                nc.tensor.matmul(py[:, 512 * co:512 * (co + 1)],
                                 attnT[m][:, 128 * nb:128 * (nb + 1)],
                                 wo_sb[m][:, 512 * co:512 * (co + 1)],
                                 start=(m == 0), stop=(m == NPAIR - 1),
                                 skip_group_check=True)

        def finish_oproj(py, nb, tail=False):
            ysb = y_pool.tile([128, CD], BF, tag="ysb", name="ysb")
            if tail:
                # split the PSUM evacuation across both copy engines; keep the
                # tail DMAs off gpsimd so its exit drain isn't serialized
                nc.vector.tensor_copy(ysb[:, 0:512], py[:, 0:512])
                nc.scalar.copy(ysb[:, 512:1024], py[:, 512:1024])
                dq = [nc.sync, nc.scalar]
            else:
                # mid-attention: keep the ScalarE queue pure-exp -- a scalar
                # copy (or DMA trigger) here head-of-line-blocks the exp
                # stream behind its semaphore wait on the py matmuls
                nc.vector.tensor_copy(ysb, py)
                dq = [nc.sync, nc.gpsimd]
            dq[nb % len(dq)].dma_start(y_d[128 * nb:128 * (nb + 1), :], ysb)

        def emit_oproj(nb, tag, pool=None, tail=False):
            py = (pool or pu_pool).tile([128, WQ], FP, tag=tag, name=f"py{nb}")
            for chunk in range(4):
                oproj_mms(py, nb, chunk)
            finish_oproj(py, nb, tail)

        def emit_scores(m, w, kb, sp, pt):
            heads = (2 * m, 2 * m + 1)
            i0 = 128 * (kb - 1)
            q0 = max(i0, WQ * w)
            o = q0 - WQ * w
            width = WQ * (w + 1) - q0
            diag = i0 >= WQ * w
            for hi in range(2):
                h = heads[hi]
                sp[kb, hi] = sp_pool.tile([128, WQ], FP, tag="sp", name="sp")
                c0 = q0
                while c0 < WQ * (w + 1):
                    c1 = min(512 * (c0 // 512 + 1), WQ * (w + 1))
                    co = c0 - WQ * w
                    is_diag_chunk = diag and c0 == i0
                    nc.tensor.matmul(
                        sp[kb, hi][:, co:co + (c1 - c0)],
                        qkZ[h][:, i0:i0 + 128],
                        qkT[m][:, c0:c1],
                        start=True, stop=not is_diag_chunk,
                        skip_group_check=True)
                    if is_diag_chunk:
                        nc.tensor.matmul(
                            sp[kb, hi][:, co:co + 128],
                            trinegT, identb,
                            start=False, stop=True,
                            skip_group_check=True)
                    c0 = c1
            for hi in range(2):
                pt[kb, hi] = pt_pool.tile([128, WQ], BF, tag="pt", name="pt")
                nc.scalar.activation(pt[kb, hi][:, o:o + width],
                                     sp[kb, hi][:, o:o + width],
                                     mybir.ActivationFunctionType.Exp,
                                     scale=SCALE)

        with tc.tile_pool(name="sp", bufs=2, space="PSUM") as sp_pool, \
             tc.tile_pool(name="pu", bufs=1, space="PSUM") as pu_pool, \
             tc.tile_pool(name="pt", bufs=4) as pt_pool, \
             tc.tile_pool(name="u8", bufs=3) as u8_pool, \
             tc.tile_pool(name="rbc", bufs=2) as rbc_pool, \
             tc.tile_pool(name="seed", bufs=1) as seed_pool, \
             tc.tile_pool(name="ysb", bufs=3) as y_pool:
            carry = {}
            units = [(w, m) for w in range(NW) for m in range(NPAIR)]
            for ui, (w, m) in enumerate(units):
                if True:
                    heads = (2 * m, 2 * m + 1)
                    sl = slice(WQ * w, WQ * (w + 1))
                    nkb = 8 * (w + 1)  # key blocks visible in this window
                    if (w, m) in carry:
                        # scores(1) were already issued during the previous
                        # unit's last key block (cross-unit lookahead)
                        sp, pt = carry.pop((w, m))
                    else:
                        sp, pt = {}, {}
                        emit_scores(m, w, 1, sp, pt)
                    # interleave one queued window-0 out-projection at the pair
                    # boundary, after the first scores (so the ACT stays fed)
                    # while this pair's pu banks are free (skip m=0: pair 3's
                    # window-0 norm has not drained yet there)
                    if oproj_q and m > 0:
                        nb, tag = oproj_q.pop(0)
                        emit_oproj(nb, tag)
                    pu, seedt = {}, {}
                    for hi in range(2):
                        pu[hi] = pu_pool.tile([128, WQ], FP, tag=f"pu{hi}", name=f"pu{hi}")
                    for kb in range(1, nkb + 1):
                        if kb < nkb:
                            emit_scores(m, w, kb + 1, sp, pt)
                        elif ui + 1 < len(units):
                            # cross-unit lookahead: issue the next unit's first
                            # scores before this unit's last av matmuls -- both
                            # wait on exp(nkb), so the PE loses nothing and the
                            # ACT sees no gap at the pair transition
                            wn, mn = units[ui + 1]
                            spn, ptn = {}, {}
                            emit_scores(mn, wn, 1, spn, ptn)
                            carry[(wn, mn)] = (spn, ptn)
                        q0 = max(128 * (kb - 1), WQ * w)
                        for hi in range(2):
                            h = heads[hi]
                            c0 = q0
                            while c0 < WQ * (w + 1):
                                c1 = min(512 * (c0 // 512 + 1), WQ * (w + 1))
                                co = c0 - WQ * w
                                nc.tensor.matmul(
                                    pu[hi][:, co:co + (c1 - c0)],
                                    v65[kb - 1][:, 65 * heads[hi]:65 * heads[hi] + 128],
                                    pt[kb, hi][:, co:co + (c1 - c0)],
                                    start=(kb == 1),
                                    stop=(kb == nkb and c1 == WQ * (w + 1)))
                                c0 = c1
                        # ctx (key 0) contribution: seed = v_ctx (x) pcx row,
                        # built off the PE (gpsimd broadcast + DVE scale) and
                        # added at evacuation time
                        if kb == 1:
                            for hi in range(2):
                                p0 = seed_pool.tile([1, WQ], BF, tag=f"p0_{hi}", name="p0")
                                nc.sync.dma_start(
                                    p0, pcx8[2 * m + hi:2 * m + hi + 1, sl])
                                seedt[hi] = {"p0": p0}
                        elif kb == 2:
                            for hi in range(2):
                                pb = seed_pool.tile([65, WQ], BF, tag=f"pb_{hi}", name="pb")
                                nc.gpsimd.partition_broadcast(pb, seedt[hi]["p0"])
                                sd = seed_pool.tile([65, WQ], FP, tag=f"sd_{hi}", name="sd")
                                nc.vector.tensor_scalar_mul(
                                    sd, pb, vct65[:, heads[hi]:heads[hi] + 1])
                                seedt[hi]["sd"] = sd
                        # prefetch next pair's zero-banded q^T during w=0
                        if w == 0 and kb == 5 and m + 1 < NPAIR:
                            emit_qkz(m + 1)
                        # drain one deferred normalization step per key block
                        if norm_q and kb in (3, 4, 6, 7):
                            emit_norm(norm_q.pop(0))
                    # evacuate U to SBUF (frees the PSUM accumulator quickly)
                    # while adding the ctx seed; defer the normalize chain
                    u8h = {}
                    for hi in range(2):
                        u8 = u8_pool.tile([65, WQ], FP, tag=f"u8_{hi}", name="u8")
                        nc.vector.tensor_add(u8, pu[hi][0:65, :], seedt[hi]["sd"])
                        u8h[hi] = {"u8": u8}
                    for stage in range(4):
                        norm_q.append([m, w, u8h, stage])
                if w == 0 and m == NPAIR - 1:
                    # window-0 out-projection becomes available once pair 3's
                    # norm drains; queue blocks for the window-1 boundaries
                    for nb in range(8):
                        oproj_q.append((nb, f"pu{nb % 2}"))
            while norm_q:
                emit_norm(norm_q.pop(0))
            # remaining out-projection blocks (window 1 + any leftovers),
            # rotating over 4 PSUM slots (pu tags + freed sp slots)
            left = [nb for nb, _ in oproj_q] + list(range(8, NQB))
            slots = [("pu0", pu_pool), ("pu1", pu_pool), ("sp", sp_pool), ("sp", sp_pool)]
            for i, nb in enumerate(left):
                tag, pool = slots[i % 4]
                emit_oproj(nb, tag, pool, tail=True)

    nc.compile()
    return nc


_NC = None


def _get_nc():
    global _NC
    if _NC is None:
        _NC = _build_nc()
    return _NC


def _shard(inputs):
    x = np.asarray(inputs["x"], dtype=np.float32)
    context = np.ascontiguousarray(np.asarray(inputs["context"], dtype=np.float32))
    Wq = np.asarray(inputs["Wq"], dtype=np.float32)
    Wk = np.asarray(inputs["Wk"], dtype=np.float32)
    Wv = np.asarray(inputs["Wv"], dtype=np.float32)
    Wo = np.asarray(inputs["Wo"], dtype=np.float32)
    xb = x.astype(ml_dtypes.bfloat16)
    Wqb = Wq.astype(ml_dtypes.bfloat16)
    Wob = Wo.astype(ml_dtypes.bfloat16)
    kctx = context @ Wk   # [B, 1024] host-side 1-row projections
    vctx = context @ Wv
    in_maps = []
    for c in range(8):
        b, g = c // 2, c % 2
        sl = slice(HD * g, HD * (g + 1))
        in_maps.append({
            "xt": np.ascontiguousarray(xb[b].T),
            "wq": np.ascontiguousarray(Wqb[:, sl]),
            "wo": np.ascontiguousarray(Wob[sl, :]),
            "kctx": np.ascontiguousarray(kctx[b:b + 1, sl]),
            "vctx": np.ascontiguousarray(vctx[b:b + 1, sl]),
        })
    return in_maps


def _run(inputs, trace=False, **kw):
    nc = _get_nc()
    in_maps = _shard(inputs)
    res = run_bass_kernel_spmd(nc, in_maps, list(range(8)), trace=trace, **kw)
    bo = np.asarray(inputs["bo"], dtype=np.float32)
    B = np.asarray(inputs["x"]).shape[0]
    y = np.empty((B, N, CD), dtype=np.float32)
    for b in range(B):
        y[b] = (res.results[2 * b]["y"].astype(np.float32)
                + res.results[2 * b + 1]["y"].astype(np.float32) + bo)
    return y, res


def kernel(**inputs):
    y, _ = _run(inputs)
    return y
